# revision 32
# baseline (speedup 1.0000x reference)
"""AttnReadout (segment softmax-attention readout) on 8 trn2 NeuronCores.

Math (per graph segment b, nodes n in b):
    feat_u = feats @ W_u + b_u                      [N, H]
    feat_v = (feats[last_nodes] @ W_v)[seg_ids]     [N, H]
    e      = sigmoid(feat_u + feat_v) @ w_e         [N]
    alpha  = softmax(e) within segment              [N]
    rst[b] = sum_n alpha_n * feats[n]               [B, D]

Device strategy (data-parallel over whole segments, 64 segments/core):
  * transposed-compute pipeline: feat_u^T [H, nodes] via PE with W_u as
    stationary and DMA-transposed bf16 feats streaming; feat_v broadcast
    to nodes with a one-hot(seg) matmul (b_u folded into the fv table);
    sigmoid on ACT; e via per-tile matmul with sig^T stationary and w_e
    streaming (lands partition-aligned); softmax without max-subtraction
    (|e| <= sum|w_e| <= 16, exp is fp32-safe); segment reduce as a
    matmul with an exp(e)-weighted one-hot as lhsT (fp32r) against the
    natural-layout fp32 feats, accumulated in PSUM across all tiles.
  * padding nodes carry seg_id=64 -> one-hot columns all zero -> they
    contribute nothing to any segment.
"""

import sys

sys.path.insert(0, "/opt/trn_rl_repo")

import numpy as np
import ml_dtypes

NCORES = 8
B = 512
D = 256
H = 256
SEGS = B // NCORES  # 64 segments per core
CHUNK = 512  # nodes per pipeline chunk
BF16 = ml_dtypes.bfloat16

_CACHE: dict = {}


def _build(n_pad: int, taps: bool = False):
    from concourse import bass, bacc, tile
    import concourse.mybir as mybir

    dt = mybir.dt
    Alu = mybir.AluOpType
    Act = mybir.ActivationFunctionType

    T = n_pad // 128  # node tiles
    C = n_pad // CHUNK  # chunks

    nc = bacc.Bacc("TRN2", target_bir_lowering=False, debug=False, num_devices=NCORES)

    f32 = dt.float32
    f32r = dt.float32r
    bf16 = dt.bfloat16

    feats = nc.dram_tensor("feats", [n_pad, D + 2], f32r, kind="ExternalInput").ap()
    seg_bcast = nc.dram_tensor("seg_bcast", [SEGS, n_pad], bf16, kind="ExternalInput").ap()
    seg_cols = nc.dram_tensor("seg_cols", [128, T], f32, kind="ExternalInput").ap()
    lastfT = nc.dram_tensor("lastfT", [D, SEGS], bf16, kind="ExternalInput").ap()
    Wu = nc.dram_tensor("Wu", [D, H], bf16, kind="ExternalInput").ap()
    Wv = nc.dram_tensor("Wv", [D, H], bf16, kind="ExternalInput").ap()
    we_cols = nc.dram_tensor("we_cols", [128, 2], bf16, kind="ExternalInput").ap()
    bu_bcast = nc.dram_tensor("bu_bcast", [SEGS, H], f32, kind="ExternalInput").ap()
    iota_col = nc.dram_tensor("iota_col", [SEGS, 1], f32, kind="ExternalInput").ap()
    iota_bcast = nc.dram_tensor("iota_bcast", [128, SEGS], f32, kind="ExternalInput").ap()
    ident = nc.dram_tensor("ident", [128, 128], bf16, kind="ExternalInput").ap()
    rst = nc.dram_tensor("rst", [SEGS, D], f32, kind="ExternalOutput").ap()
    if taps:
        T_ = n_pad // 128
        dbg_ex = nc.dram_tensor("dbg_ex", [128, T_], f32, kind="ExternalOutput").ap()
        dbg_sig = nc.dram_tensor("dbg_sig", [128, 2, CHUNK], bf16, kind="ExternalOutput").ap()
        dbg_fT = nc.dram_tensor("dbg_fT", [128, 2, CHUNK], bf16, kind="ExternalOutput").ap()
        dbg_ohw = nc.dram_tensor("dbg_ohw", [128, 4, SEGS], f32r, kind="ExternalOutput").ap()

    from contextlib import ExitStack

    with tile.TileContext(nc) as tc, ExitStack() as ctx:
        consts = ctx.enter_context(tc.tile_pool(name="consts", bufs=1))
        fpool = ctx.enter_context(tc.tile_pool(name="fpool", bufs=6))
        fbpool = ctx.enter_context(tc.tile_pool(name="fbpool", bufs=6))
        ftpool = ctx.enter_context(tc.tile_pool(name="ftpool", bufs=6))
        ohpool = ctx.enter_context(tc.tile_pool(name="ohpool", bufs=6))
        sigpool = ctx.enter_context(tc.tile_pool(name="sigpool", bufs=6))
        expool = ctx.enter_context(tc.tile_pool(name="expool", bufs=6))
        ohwpool = ctx.enter_context(tc.tile_pool(name="ohwpool", bufs=6))
        outpool = ctx.enter_context(tc.tile_pool(name="outpool", bufs=1))
        ppu = ctx.enter_context(tc.tile_pool(name="ppu", bufs=3, space="PSUM"))
        ppt = ctx.enter_context(tc.tile_pool(name="ppt", bufs=2, space="PSUM"))
        ppe = ctx.enter_context(tc.tile_pool(name="ppe", bufs=2, space="PSUM"))
        ppr = ctx.enter_context(tc.tile_pool(name="ppr", bufs=1, space="PSUM"))

        # ---- constants into SBUF ----
        Wu_sb = consts.tile([128, 2, H], bf16)
        nc.scalar.dma_start(out=Wu_sb[:], in_=Wu.rearrange("(k p) h -> p k h", p=128))
        Wv_sb = consts.tile([128, 2, H], bf16)
        nc.scalar.dma_start(out=Wv_sb[:], in_=Wv.rearrange("(k p) h -> p k h", p=128))
        lastfT_sb = consts.tile([128, 2, SEGS], bf16)
        nc.scalar.dma_start(
            out=lastfT_sb[:], in_=lastfT.rearrange("(k p) s -> p k s", p=128)
        )
        we_sb = consts.tile([128, 2], bf16)
        nc.scalar.dma_start(out=we_sb[:], in_=we_cols)
        bu_sb = consts.tile([SEGS, H], f32)
        nc.scalar.dma_start(out=bu_sb[:], in_=bu_bcast)
        iotac_sb = consts.tile([SEGS, 1], f32)
        nc.scalar.dma_start(out=iotac_sb[:], in_=iota_col)
        iotab_sb = consts.tile([128, SEGS], f32)
        nc.scalar.dma_start(out=iotab_sb[:], in_=iota_bcast)
        ident_sb = consts.tile([128, 128], bf16)
        nc.scalar.dma_start(out=ident_sb[:], in_=ident)
        segb_sb = consts.tile([SEGS, n_pad], bf16)
        nc.scalar.dma_start(out=segb_sb[:], in_=seg_bcast)
        segcols_sb = consts.tile([128, T], f32)
        nc.scalar.dma_start(out=segcols_sb[:], in_=seg_cols)

        # ---- fv = lastfeats @ W_v + b_u  (per-segment table, bf16) ----
        psum_fv = ppe.tile([SEGS, H], f32, tag="pe_")
        for k in range(2):
            nc.tensor.matmul(
                psum_fv[:],
                lastfT_sb[:, k, :],
                Wv_sb[:, k, :],
                start=(k == 0),
                stop=(k == 1),
            )
        fvb_sb = consts.tile([SEGS, H], bf16)
        nc.vector.tensor_tensor(
            out=fvb_sb[:], in0=psum_fv[:], in1=bu_sb[:], op=Alu.add
        )

        # ---- global reduction accumulator (cols 0..D-1 = num, col D = den) ----
        psum_r = ppr.tile([SEGS, D + 2], f32)

        # ---- streaming over chunks of 512 nodes (software-pipelined:
        # chunk c+1's load/transpose is emitted before chunk c's compute) ----
        def load_stage(c):
            nsl = slice(c * CHUNK, (c + 1) * CHUNK)
            ffull = fpool.tile([128, 4, D + 2], f32r)
            nc.sync.dma_start(
                out=ffull[:], in_=feats[nsl, :].rearrange("(t p) d -> p t d", p=128)
            )
            fbf = fbpool.tile([128, 4, D], bf16)
            nc.gpsimd.tensor_copy(fbf[:], ffull[:, :, 0:D].bitcast(f32))
            fT = ftpool.tile([128, 2, CHUNK], bf16)
            for h in range(2):
                pt = ppt.tile([128, CHUNK], bf16)
                for t in range(4):
                    nc.tensor.transpose(
                        pt[:, 128 * t : 128 * (t + 1)],
                        fbf[:, t, 128 * h : 128 * (h + 1)],
                        ident_sb[:],
                    )
                nc.vector.tensor_copy(fT[:, h, :], pt[:])
            oh = ohpool.tile([SEGS, CHUNK], bf16)
            nc.vector.tensor_scalar(
                out=oh[:],
                in0=segb_sb[:, c * CHUNK : (c + 1) * CHUNK],
                scalar1=iotac_sb[:, 0:1],
                scalar2=None,
                op0=Alu.is_equal,
            )
            return ffull, fT, oh

        def compute_stage(c, ffull, fT, oh):
            sigT = sigpool.tile([128, 2, CHUNK], bf16)
            for h in range(2):
                pu = ppu.tile([128, CHUNK], f32)
                nc.tensor.matmul(
                    pu[:],
                    Wu_sb[:, 0, 128 * h : 128 * (h + 1)],
                    fT[:, 0, :],
                    start=True,
                    stop=False,
                )
                nc.tensor.matmul(
                    pu[:],
                    Wu_sb[:, 1, 128 * h : 128 * (h + 1)],
                    fT[:, 1, :],
                    start=False,
                    stop=False,
                )
                nc.tensor.matmul(
                    pu[:],
                    fvb_sb[:, 128 * h : 128 * (h + 1)],
                    oh[:],
                    start=False,
                    stop=True,
                )
                nc.scalar.activation(sigT[:, h, :], pu[:], Act.Sigmoid)

            pe_ = ppe.tile([128, 4], f32, tag="pe_")
            for t in range(4):
                for k in range(2):
                    nc.tensor.matmul(
                        pe_[:, t : t + 1],
                        sigT[:, k, 128 * t : 128 * (t + 1)],
                        we_sb[:, k : k + 1],
                        start=(k == 0),
                        stop=(k == 1),
                    )
            es = expool.tile([128, 4], f32, tag="es")
            nc.scalar.activation(es[:], pe_[:], Act.Sigmoid)
            oms = expool.tile([128, 4], f32, tag="oms")
            nc.vector.tensor_scalar(
                out=oms[:], in0=es[:], scalar1=-1.0, scalar2=1.0,
                op0=Alu.mult, op1=Alu.add,
            )
            rcp = expool.tile([128, 4], f32, tag="rcp")
            nc.vector.reciprocal(rcp[:], oms[:])
            ex = expool.tile([128, 4], f32)
            nc.vector.tensor_tensor(out=ex[:], in0=es[:], in1=rcp[:], op=Alu.mult)

            ohw = ohwpool.tile([128, 4, SEGS], f32r)
            for t in range(4):
                nc.vector.tensor_scalar(
                    out=ohw[:, t, :],
                    in0=iotab_sb[:],
                    scalar1=segcols_sb[:, 4 * c + t : 4 * c + t + 1],
                    scalar2=ex[:, t : t + 1],
                    op0=Alu.is_equal,
                    op1=Alu.mult,
                )
            for t in range(4):
                first = c == 0 and t == 0
                last = c == C - 1 and t == 3
                nc.tensor.matmul(
                    psum_r[:],
                    ohw[:, t, :],
                    ffull[:, t, :],
                    start=first,
                    stop=last,
                )

        pending = None
        for c in range(C + 1):
            cur = (c, *load_stage(c)) if c < C else None
            if pending is not None:
                compute_stage(*pending)
            pending = cur

        # ---- rst = num / den ----
        den = outpool.tile([SEGS, 1], f32)
        nc.vector.tensor_scalar(
            out=den[:],
            in0=psum_r[:, D : D + 1],
            scalar1=1e-30,
            scalar2=None,
            op0=Alu.add,
        )
        rec = outpool.tile([SEGS, 1], f32)
        nc.vector.reciprocal(rec[:], den[:])
        out_sb = outpool.tile([SEGS, D], f32)
        nc.vector.tensor_scalar(
            out=out_sb[:],
            in0=psum_r[:, 0:D],
            scalar1=rec[:, 0:1],
            scalar2=None,
            op0=Alu.mult,
        )
        nc.sync.dma_start(out=rst, in_=out_sb[:])

    nc.compile()
    return nc


def _prep(inputs):
    feats = np.ascontiguousarray(np.asarray(inputs["feats_s1"], dtype=np.float32))
    last_nodes = np.asarray(inputs["last_nodes"]).astype(np.int64)
    seg_ids = np.asarray(inputs["seg_ids"]).astype(np.int64)
    W_u = np.asarray(inputs["W_u"], dtype=np.float32)
    b_u = np.asarray(inputs["b_u"], dtype=np.float32)
    W_v = np.asarray(inputs["W_v"], dtype=np.float32)
    w_e = np.asarray(inputs["w_e"], dtype=np.float32)

    bnds = np.searchsorted(seg_ids, np.arange(B + 1), side="left")
    los = bnds[0 : B : SEGS]
    his = bnds[SEGS : B + 1 : SEGS]
    counts = his - los
    n_pad = int(max(CHUNK, -(-counts.max() // CHUNK) * CHUNK))

    Wu_bf = W_u.astype(BF16)
    Wv_bf = W_v.astype(BF16)
    we_cols = np.ascontiguousarray(w_e.reshape(2, 128).T).astype(BF16)
    bu_bcast = np.tile(b_u.reshape(1, H), (SEGS, 1))
    iota_col = np.arange(SEGS, dtype=np.float32).reshape(SEGS, 1)
    iota_bcast = np.tile(np.arange(SEGS, dtype=np.float32).reshape(1, SEGS), (128, 1))
    ident = np.eye(128, dtype=np.float32).astype(BF16)

    in_maps = []
    for c in range(NCORES):
        lo, hi = int(los[c]), int(his[c])
        n = hi - lo
        fp = np.zeros((n_pad, D + 2), dtype=np.float32)
        fp[:n, :D] = feats[lo:hi]
        fp[:, D] = 1.0
        segloc = np.full(n_pad, float(SEGS), dtype=np.float32)
        segloc[:n] = (seg_ids[lo:hi] - SEGS * c).astype(np.float32)
        lastfT = np.ascontiguousarray(
            feats[last_nodes[SEGS * c : SEGS * (c + 1)]].T
        ).astype(BF16)
        in_maps.append(
            {
                "feats": fp,
                "seg_bcast": np.tile(segloc.reshape(1, n_pad), (SEGS, 1)).astype(BF16),
                "seg_cols": np.ascontiguousarray(segloc.reshape(-1, 128).T),
                "lastfT": lastfT,
                "Wu": Wu_bf,
                "Wv": Wv_bf,
                "we_cols": we_cols,
                "bu_bcast": bu_bcast,
                "iota_col": iota_col,
                "iota_bcast": iota_bcast,
                "ident": ident,
            }
        )
    return in_maps, n_pad


def _run(inputs, trace=False, trace_kwargs=None, taps=False):
    from concourse import bass_utils

    in_maps, n_pad = _prep(inputs)
    key = (n_pad, bool(taps))
    if key not in _CACHE:
        _CACHE[key] = _build(n_pad, taps=taps)
    nc = _CACHE[key]
    res = bass_utils.run_bass_kernel_spmd(
        nc,
        in_maps,
        list(range(NCORES)),
        trace=trace,
        **(trace_kwargs or {}),
    )
    out = np.concatenate([res.results[c]["rst"] for c in range(NCORES)], axis=0)
    return out[:, None, :].astype(np.float32), res


def kernel(**inputs) -> np.ndarray:
    return _run(inputs)[0]


# revision 36
# speedup vs baseline: 1.0370x; 1.0370x over previous
"""AttnReadout (segment softmax-attention readout) on 8 trn2 NeuronCores.

Math (per graph segment b, nodes n in b):
    feat_u = feats @ W_u + b_u                      [N, H]
    feat_v = (feats[last_nodes] @ W_v)[seg_ids]     [N, H]
    e      = sigmoid(feat_u + feat_v) @ w_e         [N]
    alpha  = softmax(e) within segment              [N]
    rst[b] = sum_n alpha_n * feats[n]               [B, D]

Device strategy (data-parallel over whole segments, 64 segments/core):
  * transposed-compute pipeline: feat_u^T [H, nodes] via PE with W_u as
    stationary and DMA-transposed bf16 feats streaming; feat_v broadcast
    to nodes with a one-hot(seg) matmul (b_u folded into the fv table);
    sigmoid on ACT; e via per-tile matmul with sig^T stationary and w_e
    streaming (lands partition-aligned); softmax without max-subtraction
    (|e| <= sum|w_e| <= 16, exp is fp32-safe); segment reduce as a
    matmul with an exp(e)-weighted one-hot as lhsT (fp32r) against the
    natural-layout fp32 feats, accumulated in PSUM across all tiles.
  * padding nodes carry seg_id=64 -> one-hot columns all zero -> they
    contribute nothing to any segment.
"""

import sys

sys.path.insert(0, "/opt/trn_rl_repo")

import numpy as np
import ml_dtypes

NCORES = 8
B = 512
D = 256
H = 256
SEGS = B // NCORES  # 64 segments per core
CHUNK = 512  # nodes per pipeline chunk
BF16 = ml_dtypes.bfloat16

_CACHE: dict = {}


def _build(n_pad: int, taps: bool = False):
    from concourse import bass, bacc, tile
    import concourse.mybir as mybir

    dt = mybir.dt
    Alu = mybir.AluOpType
    Act = mybir.ActivationFunctionType

    T = n_pad // 128  # node tiles
    C = n_pad // CHUNK  # chunks

    nc = bacc.Bacc("TRN2", target_bir_lowering=False, debug=False, num_devices=NCORES)

    f32 = dt.float32
    f32r = dt.float32r
    bf16 = dt.bfloat16

    feats = nc.dram_tensor("feats", [n_pad, D + 2], f32r, kind="ExternalInput").ap()
    seg_bcast = nc.dram_tensor("seg_bcast", [SEGS, n_pad], bf16, kind="ExternalInput").ap()
    seg_cols = nc.dram_tensor("seg_cols", [128, T], f32, kind="ExternalInput").ap()
    lastfT = nc.dram_tensor("lastfT", [D, SEGS], bf16, kind="ExternalInput").ap()
    Wu = nc.dram_tensor("Wu", [D, H], bf16, kind="ExternalInput").ap()
    Wv = nc.dram_tensor("Wv", [D, H], bf16, kind="ExternalInput").ap()
    we_cols = nc.dram_tensor("we_cols", [128, 2], bf16, kind="ExternalInput").ap()
    bu_bcast = nc.dram_tensor("bu_bcast", [SEGS, H], f32, kind="ExternalInput").ap()
    iota_col = nc.dram_tensor("iota_col", [SEGS, 1], f32, kind="ExternalInput").ap()
    iota_bcast = nc.dram_tensor("iota_bcast", [128, SEGS], f32, kind="ExternalInput").ap()
    ident = nc.dram_tensor("ident", [128, 128], bf16, kind="ExternalInput").ap()
    rst = nc.dram_tensor("rst", [SEGS, D], f32, kind="ExternalOutput").ap()
    if taps:
        T_ = n_pad // 128
        dbg_ex = nc.dram_tensor("dbg_ex", [128, T_], f32, kind="ExternalOutput").ap()
        dbg_sig = nc.dram_tensor("dbg_sig", [128, 2, CHUNK], bf16, kind="ExternalOutput").ap()
        dbg_fT = nc.dram_tensor("dbg_fT", [128, 2, CHUNK], bf16, kind="ExternalOutput").ap()
        dbg_ohw = nc.dram_tensor("dbg_ohw", [128, 4, SEGS], f32r, kind="ExternalOutput").ap()

    from contextlib import ExitStack

    with tile.TileContext(nc) as tc, ExitStack() as ctx:
        consts = ctx.enter_context(tc.tile_pool(name="consts", bufs=1))
        fpool = ctx.enter_context(tc.tile_pool(name="fpool", bufs=6))
        fbpool = ctx.enter_context(tc.tile_pool(name="fbpool", bufs=6))
        ftpool = ctx.enter_context(tc.tile_pool(name="ftpool", bufs=6))
        ohpool = ctx.enter_context(tc.tile_pool(name="ohpool", bufs=6))
        sigpool = ctx.enter_context(tc.tile_pool(name="sigpool", bufs=6))
        expool = ctx.enter_context(tc.tile_pool(name="expool", bufs=6))
        ohwpool = ctx.enter_context(tc.tile_pool(name="ohwpool", bufs=6))
        outpool = ctx.enter_context(tc.tile_pool(name="outpool", bufs=1))
        ppu = ctx.enter_context(tc.tile_pool(name="ppu", bufs=3, space="PSUM"))
        ppt = ctx.enter_context(tc.tile_pool(name="ppt", bufs=2, space="PSUM"))
        ppe = ctx.enter_context(tc.tile_pool(name="ppe", bufs=2, space="PSUM"))
        ppr = ctx.enter_context(tc.tile_pool(name="ppr", bufs=1, space="PSUM"))

        # ---- constants into SBUF (issue order matters: HWDGE generates
        # descriptors serially at ~630ns/DMA, so loop-critical deps first) ----
        ident_sb = consts.tile([128, 128], bf16)
        nc.scalar.dma_start(out=ident_sb[:], in_=ident)
        Wu_sb = consts.tile([128, 2, H], bf16)
        nc.scalar.dma_start(out=Wu_sb[:], in_=Wu.rearrange("(k p) h -> p k h", p=128))
        Wv_sb = consts.tile([128, 2, H], bf16)
        nc.scalar.dma_start(out=Wv_sb[:], in_=Wv.rearrange("(k p) h -> p k h", p=128))
        lastfT_sb = consts.tile([128, 2, SEGS], bf16)
        nc.scalar.dma_start(
            out=lastfT_sb[:], in_=lastfT.rearrange("(k p) s -> p k s", p=128)
        )
        segb_sb = consts.tile([SEGS, n_pad], bf16)
        nc.scalar.dma_start(out=segb_sb[:, 0:CHUNK], in_=seg_bcast[:, 0:CHUNK])
        iotac_sb = consts.tile([SEGS, 1], f32)
        nc.scalar.dma_start(out=iotac_sb[:], in_=iota_col)
        we_sb = consts.tile([128, 2], bf16)
        nc.scalar.dma_start(out=we_sb[:], in_=we_cols)
        bu_sb = consts.tile([SEGS, H], f32)
        nc.scalar.dma_start(out=bu_sb[:], in_=bu_bcast)
        iotab_sb = consts.tile([128, SEGS], f32)
        nc.scalar.dma_start(out=iotab_sb[:], in_=iota_bcast)
        segcols_sb = consts.tile([128, T], f32)
        nc.scalar.dma_start(out=segcols_sb[:], in_=seg_cols)
        nc.scalar.dma_start(out=segb_sb[:, CHUNK:], in_=seg_bcast[:, CHUNK:])

        # ---- fv = lastfeats @ W_v + b_u  (per-segment table, bf16) ----
        psum_fv = ppe.tile([SEGS, H], f32, tag="pe_")
        for k in range(2):
            nc.tensor.matmul(
                psum_fv[:],
                lastfT_sb[:, k, :],
                Wv_sb[:, k, :],
                start=(k == 0),
                stop=(k == 1),
            )
        fvb_sb = consts.tile([SEGS, H], bf16)
        nc.vector.tensor_tensor(
            out=fvb_sb[:], in0=psum_fv[:], in1=bu_sb[:], op=Alu.add
        )

        # ---- global reduction accumulator (cols 0..D-1 = num, col D = den) ----
        psum_r = ppr.tile([SEGS, D + 2], f32)

        # ---- streaming over chunks of 512 nodes (software-pipelined:
        # chunk c+1's load/transpose is emitted before chunk c's compute) ----
        def load_stage(c):
            nsl = slice(c * CHUNK, (c + 1) * CHUNK)
            ffull = fpool.tile([128, 4, D + 2], f32r)
            nc.sync.dma_start(
                out=ffull[:], in_=feats[nsl, :].rearrange("(t p) d -> p t d", p=128)
            )
            fbf = fbpool.tile([128, 4, D], bf16)
            nc.gpsimd.tensor_copy(fbf[:], ffull[:, :, 0:D].bitcast(f32))
            fT = ftpool.tile([128, 2, CHUNK], bf16)
            for h in range(2):
                pt = ppt.tile([128, CHUNK], bf16)
                for t in range(4):
                    nc.tensor.transpose(
                        pt[:, 128 * t : 128 * (t + 1)],
                        fbf[:, t, 128 * h : 128 * (h + 1)],
                        ident_sb[:],
                    )
                nc.vector.tensor_copy(fT[:, h, :], pt[:])
            oh = ohpool.tile([SEGS, CHUNK], bf16)
            nc.vector.tensor_scalar(
                out=oh[:],
                in0=segb_sb[:, c * CHUNK : (c + 1) * CHUNK],
                scalar1=iotac_sb[:, 0:1],
                scalar2=None,
                op0=Alu.is_equal,
            )
            return ffull, fT, oh

        def compute_stage(c, ffull, fT, oh):
            sigT = sigpool.tile([128, 2, CHUNK], bf16)
            for h in range(2):
                pu = ppu.tile([128, CHUNK], f32)
                nc.tensor.matmul(
                    pu[:],
                    Wu_sb[:, 0, 128 * h : 128 * (h + 1)],
                    fT[:, 0, :],
                    start=True,
                    stop=False,
                )
                nc.tensor.matmul(
                    pu[:],
                    Wu_sb[:, 1, 128 * h : 128 * (h + 1)],
                    fT[:, 1, :],
                    start=False,
                    stop=False,
                )
                nc.tensor.matmul(
                    pu[:],
                    fvb_sb[:, 128 * h : 128 * (h + 1)],
                    oh[:],
                    start=False,
                    stop=True,
                )
                nc.scalar.activation(sigT[:, h, :], pu[:], Act.Sigmoid)

            pe_ = ppe.tile([128, 4], f32, tag="pe_")
            for t in range(4):
                for k in range(2):
                    nc.tensor.matmul(
                        pe_[:, t : t + 1],
                        sigT[:, k, 128 * t : 128 * (t + 1)],
                        we_sb[:, k : k + 1],
                        start=(k == 0),
                        stop=(k == 1),
                    )
            es = expool.tile([128, 4], f32, tag="es")
            nc.scalar.activation(es[:], pe_[:], Act.Sigmoid)
            oms = expool.tile([128, 4], f32, tag="oms")
            nc.vector.tensor_scalar(
                out=oms[:], in0=es[:], scalar1=-1.0, scalar2=1.0,
                op0=Alu.mult, op1=Alu.add,
            )
            rcp = expool.tile([128, 4], f32, tag="rcp")
            nc.vector.reciprocal(rcp[:], oms[:])
            ex = expool.tile([128, 4], f32)
            nc.vector.tensor_tensor(out=ex[:], in0=es[:], in1=rcp[:], op=Alu.mult)

            ohw = ohwpool.tile([128, 4, SEGS], f32r)
            for t in range(4):
                nc.vector.tensor_scalar(
                    out=ohw[:, t, :],
                    in0=iotab_sb[:],
                    scalar1=segcols_sb[:, 4 * c + t : 4 * c + t + 1],
                    scalar2=ex[:, t : t + 1],
                    op0=Alu.is_equal,
                    op1=Alu.mult,
                )
            for t in range(4):
                first = c == 0 and t == 0
                last = c == C - 1 and t == 3
                nc.tensor.matmul(
                    psum_r[:],
                    ohw[:, t, :],
                    ffull[:, t, :],
                    start=first,
                    stop=last,
                )

        pending = None
        for c in range(C + 1):
            cur = (c, *load_stage(c)) if c < C else None
            if pending is not None:
                compute_stage(*pending)
            pending = cur

        # ---- rst = num / den ----
        den = outpool.tile([SEGS, 1], f32)
        nc.vector.tensor_scalar(
            out=den[:],
            in0=psum_r[:, D : D + 1],
            scalar1=1e-30,
            scalar2=None,
            op0=Alu.add,
        )
        rec = outpool.tile([SEGS, 1], f32)
        nc.vector.reciprocal(rec[:], den[:])
        out_sb = outpool.tile([SEGS, D], f32)
        nc.vector.tensor_scalar(
            out=out_sb[:],
            in0=psum_r[:, 0:D],
            scalar1=rec[:, 0:1],
            scalar2=None,
            op0=Alu.mult,
        )
        nc.sync.dma_start(out=rst, in_=out_sb[:])

    nc.compile()
    return nc


def _prep(inputs):
    feats = np.ascontiguousarray(np.asarray(inputs["feats_s1"], dtype=np.float32))
    last_nodes = np.asarray(inputs["last_nodes"]).astype(np.int64)
    seg_ids = np.asarray(inputs["seg_ids"]).astype(np.int64)
    W_u = np.asarray(inputs["W_u"], dtype=np.float32)
    b_u = np.asarray(inputs["b_u"], dtype=np.float32)
    W_v = np.asarray(inputs["W_v"], dtype=np.float32)
    w_e = np.asarray(inputs["w_e"], dtype=np.float32)

    bnds = np.searchsorted(seg_ids, np.arange(B + 1), side="left")
    los = bnds[0 : B : SEGS]
    his = bnds[SEGS : B + 1 : SEGS]
    counts = his - los
    n_pad = int(max(CHUNK, -(-counts.max() // CHUNK) * CHUNK))

    Wu_bf = W_u.astype(BF16)
    Wv_bf = W_v.astype(BF16)
    we_cols = np.ascontiguousarray(w_e.reshape(2, 128).T).astype(BF16)
    bu_bcast = np.tile(b_u.reshape(1, H), (SEGS, 1))
    iota_col = np.arange(SEGS, dtype=np.float32).reshape(SEGS, 1)
    iota_bcast = np.tile(np.arange(SEGS, dtype=np.float32).reshape(1, SEGS), (128, 1))
    ident = np.eye(128, dtype=np.float32).astype(BF16)

    in_maps = []
    for c in range(NCORES):
        lo, hi = int(los[c]), int(his[c])
        n = hi - lo
        fp = np.zeros((n_pad, D + 2), dtype=np.float32)
        fp[:n, :D] = feats[lo:hi]
        fp[:, D] = 1.0
        segloc = np.full(n_pad, float(SEGS), dtype=np.float32)
        segloc[:n] = (seg_ids[lo:hi] - SEGS * c).astype(np.float32)
        lastfT = np.ascontiguousarray(
            feats[last_nodes[SEGS * c : SEGS * (c + 1)]].T
        ).astype(BF16)
        in_maps.append(
            {
                "feats": fp,
                "seg_bcast": np.tile(segloc.reshape(1, n_pad), (SEGS, 1)).astype(BF16),
                "seg_cols": np.ascontiguousarray(segloc.reshape(-1, 128).T),
                "lastfT": lastfT,
                "Wu": Wu_bf,
                "Wv": Wv_bf,
                "we_cols": we_cols,
                "bu_bcast": bu_bcast,
                "iota_col": iota_col,
                "iota_bcast": iota_bcast,
                "ident": ident,
            }
        )
    return in_maps, n_pad


def _run(inputs, trace=False, trace_kwargs=None, taps=False):
    from concourse import bass_utils

    in_maps, n_pad = _prep(inputs)
    key = (n_pad, bool(taps))
    if key not in _CACHE:
        _CACHE[key] = _build(n_pad, taps=taps)
    nc = _CACHE[key]
    res = bass_utils.run_bass_kernel_spmd(
        nc,
        in_maps,
        list(range(NCORES)),
        trace=trace,
        **(trace_kwargs or {}),
    )
    out = np.concatenate([res.results[c]["rst"] for c in range(NCORES)], axis=0)
    return out[:, None, :].astype(np.float32), res


def kernel(**inputs) -> np.ndarray:
    return _run(inputs)[0]
